# revision 17
# baseline (speedup 1.0000x reference)
"""Trainium2 Bass kernel for GQA attention (nn_Attention_43181601194655).

Full module: hidden [B,S,HID] -> Wq/Wk/Wv projections -> RoPE -> causal GQA
attention -> Wo projection. Tensor-parallel over heads across 8 NeuronCores
(per the TP sharding hint): core c owns q-heads [4c..4c+4) and kv-head c
(Wq/Wk/Wv column slices, Wo row slice). Each core computes a full-shape
bf16 partial output; the host sums the 8 partials (the row-parallel Wo
all-reduce) in fp32.

Per-core design (everything contracts on SBUF partitions, all matmuls bf16):
- hidden^T is pre-transposed/cast on host; streams in as [128, C, 512] tiles.
- Q^T/K^T produced directly by projection matmuls as [d, s]; per-head-pair
  PSUM tiles staged to bf16 SBUF by ACT/GpSimd (fast PSUM release), a tiny PE
  matmul against a signed permutation does rotate_half, two DVE multiplies +
  an add finish RoPE. K^T is stored zero-padded in two 128-row variants so
  score matmuls contract the full 128 PE rows.
- V^T is PE-transposed to V tiles padded to 128 lhsT columns: col 64 = ones
  (softmax denominator row), 65:128 = zeros.
- Scores are computed transposed, S^T[k,q], two heads sharing each PSUM tile;
  exp runs on ScalarE straight from PSUM with the 1/sqrt(D) scale fused.
  Causality: k-tiles above the diagonal are skipped by loop structure; the
  single diagonal 128-block gets one additive-mask DVE op.
- The score->exp->PV chain is software-pipelined at depth 2: the PE emits
  score(kt+1) and score(kt+2) before pv(kt), so exp latency (ScalarE) never
  stalls the PE.
- PSUM is partitioned into three dedicated rings so pipeline stages don't
  serialize on buffer reuse: "sc" 2x4KB (score tiles + O-proj accum),
  "pj" 2x2KB (projection psums + rotate matmuls), "pv" 2x2KB (PV
  accumulators + V-transpose tiles).
- PV uses V as stationary weights and P^T as wide-N moving data with
  causally-trimmed column ranges accumulating in PSUM; PSUM row 64 collects
  the softmax denominators. reciprocal_approx_fast + GpSimd
  partition_broadcast + one DVE multiply normalize and write attn^T[hd, q].
  Norm emission is deferred past the next head-pair's first scores so its
  latency is covered by PE work.
- Wo matmuls consume attn^T directly; PSUM->SBUF staging copies run on
  GpSimd (otherwise idle); partials go out as bf16.
- Emission is lag-interleaved (proj ss | attention ss-1 | O-proj ss-2).
"""

import sys

if "/opt/trn_rl_repo" not in sys.path:
    sys.path.insert(0, "/opt/trn_rl_repo")

import numpy as np
import ml_dtypes

import concourse.bass as bass
from concourse import bacc
import concourse.mybir as mybir
from concourse.tile import TileContext
from concourse.masks import make_identity

BF16 = mybir.dt.bfloat16
F32 = mybir.dt.float32

B, S, HID = 2, 2048, 2048
H, HKV, D = 32, 8, 64
NCORES = 8
HQ = H // NCORES          # q heads per core (4)
HD = HQ * D               # 256: per-core attn feature dim
SCALE = D ** -0.5
SSUP = 512                # q supertile width
NEG = -1e9


def build_nc(b_sz=B, s_sz=S, hid=HID):
    """Build the per-core Bass program. Parameterized for small-sim testing."""
    C = hid // 128            # contraction chunks
    n_st = s_sz // 128        # 128-tiles along s
    sup = min(SSUP, s_sz)
    n_sup = s_sz // sup
    n_qt = sup // 128         # q-tiles per supertile
    n_cs = hid // 512         # 512-wide output column chunks

    nc = bacc.Bacc()
    n_tiles = b_sz * s_sz // sup
    hsT = nc.dram_tensor("hsT", [n_tiles, 128, C, sup],
                         BF16, kind="ExternalInput")
    wq = nc.dram_tensor("wq", [128, C * HQ * D], BF16,
                        kind="ExternalInput")
    wkv = nc.dram_tensor("wkv", [128, hid], BF16, kind="ExternalInput")
    wo = nc.dram_tensor("wo", [128, HD // 128 * hid], BF16,
                        kind="ExternalInput")
    cos2 = nc.dram_tensor("cos2", [128, s_sz], BF16, kind="ExternalInput")
    sinx = nc.dram_tensor("sinx", [128, s_sz], BF16, kind="ExternalInput")
    maskd = nc.dram_tensor("maskd", [128, 128], F32, kind="ExternalInput")
    pi2d = nc.dram_tensor("pi2d", [128, 128], BF16, kind="ExternalInput")
    out = nc.dram_tensor("out", [b_sz * s_sz, hid], BF16, kind="ExternalOutput")

    wq_v = wq.rearrange("p (co m) -> p co m", co=C)
    wkv_v = wkv.rearrange("p (co m) -> p co m", co=C)
    wo_v = wo.rearrange("p (j n) -> p j n", j=HD // 128)

    with TileContext(nc) as tc:
        with (
            tc.tile_pool(name="const", bufs=1) as cpool,
            tc.tile_pool(name="hst", bufs=2) as hpool,
            tc.tile_pool(name="perb", bufs=2) as bpool,
            tc.tile_pool(name="pt", bufs=4) as ptpool,
            tc.tile_pool(name="work", bufs=2) as wpool,
            tc.tile_pool(name="outsb", bufs=2) as opool,
            tc.tile_pool(name="psum_sc", bufs=2, space="PSUM") as scpool,
            tc.tile_pool(name="psum_pj", bufs=2, space="PSUM") as pjpool,
            tc.tile_pool(name="psum_pv", bufs=2, space="PSUM") as pvpool,
        ):
            # ---- constants. sync queue: mask/pi2/hst/cos/sin (cos/sin
            # ride sync so they don't delay wq on the scalar queue);
            # scalar queue: wq (cc-chunked so the first Q matmul starts
            # after one chunk) + wkv, wo deferred. ----
            # mask/pi2/cos/sin deferred until after the first hidden
            # supertile's chunks (they aren't needed until RoPE/attention)
            # so the sync queue delivers the first proj inputs sooner
            mask_t = cpool.tile([128, 128], F32, tag="mask")
            pi2 = cpool.tile([128, 128], BF16, tag="pi2")
            # tiny first transfer doubles as DMA warm-up for the queue
            nc.sync.dma_start(pi2[:], pi2d[:])
            wq_t = cpool.tile([128, C, HQ * D], BF16, tag="wq")
            cgrp = max(1, C // 4)
            for cg in range(0, C, cgrp):
                ce = min(C, cg + cgrp)
                nc.scalar.dma_start(wq_t[:, cg:ce, :], wq_v[:, cg:ce, :])
            wkv_t = cpool.tile([128, C, 128], BF16, tag="wkv")
            nc.scalar.dma_start(wkv_t[:], wkv_v[:])
            ident = cpool.tile([128, 128], BF16, tag="ident")
            make_identity(nc, ident[:])
            cos_t = cpool.tile([128, s_sz], BF16, tag="cos")
            sin_t = cpool.tile([128, s_sz], BF16, tag="sin")
            wo_t = cpool.tile([128, HD // 128, hid], BF16, tag="wo")
            deferred = []

            def load_deferred():
                # cos/sin/mask on the Sync queue (parallel with scalar's
                # wq), emitted after the first hidden supertile
                nc.sync.dma_start(cos_t[:], cos2[:])
                nc.sync.dma_start(sin_t[:], sinx[:])
                nc.sync.dma_start(mask_t[:], maskd[:])
                nc.scalar.dma_start(wo_t[:], wo_v[:])

            def rope_rest(dst, raw, s0, rows):
                """dst[bf16 SBUF [rows,sup]] = RoPE of staged raw.

                A tiny PE matmul with the signed-permutation pi2 does
                rotate_half, two DVE multiplies + one add finish:
                dst = raw*cos + rot*sin. rot rides the "sc" ring (free at
                proj emission time) so the "pj" ring never waits on it.
                """
                rot = scpool.tile([128, sup], F32, tag="sc", name="rot")
                nc.tensor.matmul(rot[:rows, :], pi2[:, :rows], raw[:],
                                 start=True, stop=True)
                u = wpool.tile([128, sup], F32, tag="rope_u")
                t = wpool.tile([128, sup], F32, tag="rope_t")
                nc.vector.tensor_tensor(
                    u[:rows, :], raw[:rows, :], cos_t[:rows, s0:s0 + sup],
                    mybir.AluOpType.mult)
                nc.vector.tensor_tensor(
                    t[:rows, :], rot[:rows, :], sin_t[:rows, s0:s0 + sup],
                    mybir.AluOpType.mult)
                nc.vector.tensor_tensor(
                    dst, u[:rows, :], t[:rows, :], mybir.AluOpType.add)

            # both batches' persistent tiles up front so the zero/ones
            # memsets run during startup DMA instead of at the batch seam
            batch_tiles = []
            for b in range(b_sz):
                qt_b = bpool.tile([128, HQ // 2, s_sz], BF16, tag="qt",
                                  name=f"qt{b}")
                kt_b = bpool.tile([128, 2, s_sz], BF16, tag="kt",
                                  name=f"kt{b}")
                vt_b = bpool.tile([64, s_sz], BF16, tag="vt", name=f"vt{b}")
                v_b = bpool.tile([128, n_st, 128], BF16, tag="v",
                                 name=f"v{b}")
                attnT_b = bpool.tile([128, HD // 128, s_sz], BF16,
                                     tag="attnT", name=f"attnT{b}")
                nc.vector.memset(v_b[:, :, 64:65], 1.0)
                nc.vector.memset(v_b[:, :, 65:128], 0.0)
                nc.vector.memset(kt_b[64:128, 0, :], 0.0)
                nc.vector.memset(kt_b[0:64, 1, :], 0.0)
                batch_tiles.append((qt_b, kt_b, vt_b, v_b, attnT_b))

            for b in range(b_sz):
                qt_b, kt_b, vt_b, v_b, attnT_b = batch_tiles[b]

                def proj_a(ss):
                    """QKV matmuls + q-head RoPE. The kv RoPE tail lives in
                    proj_b so vtrans(ss-1) PE work can cover the GpSimd
                    rawkv staging latency."""
                    s0 = ss * sup
                    hst = hpool.tile([128, C, sup], BF16, tag="hst",
                                     name="hst")
                    for cg in range(0, C, cgrp):
                        ce = min(C, cg + cgrp)
                        nc.sync.dma_start(hst[:, cg:ce, :],
                                          hsT[b * n_sup + ss, :, cg:ce, :])
                    if not deferred:
                        deferred.append(1)
                        load_deferred()
                    # Q projection per head-pair; ACT stages PSUM->bf16
                    # immediately (fast "pj" ring release)
                    raws = []
                    for hp in range(HQ // 2):
                        psq = pjpool.tile([128, sup], F32, tag="pj",
                                          name="psq")
                        for cc in range(C):
                            nc.tensor.matmul(
                                psq[:], wq_t[:, cc, hp * 128:(hp + 1) * 128],
                                hst[:, cc, :],
                                start=(cc == 0), stop=(cc == C - 1))
                        raw = wpool.tile([128, sup], BF16, tag="rope_raw",
                                         name="raw", bufs=4)
                        nc.scalar.copy(raw[:], psq[:])
                        raws.append(raw)
                    # rotate matmul for hp0 fills PE while KV proj's raw
                    # staging completes
                    rope_rest(qt_b[:, 0, s0:s0 + sup], raws[0], s0, 128)
                    pskv = pjpool.tile([128, sup], F32, tag="pj",
                                       name="pskv")
                    for cc in range(C):
                        nc.tensor.matmul(
                            pskv[:], wkv_t[:, cc, :], hst[:, cc, :],
                            start=(cc == 0), stop=(cc == C - 1))
                    rawkv = wpool.tile([128, sup], BF16, tag="rope_raw",
                                       name="rawkv", bufs=4)
                    nc.scalar.copy(rawkv[:], pskv[:])
                    rope_rest(qt_b[:, 1, s0:s0 + sup], raws[1], s0, 128)
                    return rawkv

                def proj_b(ss, rawkv):
                    s0 = ss * sup
                    rope_rest(kt_b[:64, 0, s0:s0 + sup], rawkv, s0, 64)
                    nc.vector.tensor_copy(
                        kt_b[64:128, 1, s0:s0 + sup], kt_b[:64, 0, s0:s0 + sup])
                    nc.vector.tensor_copy(
                        vt_b[:, s0:s0 + sup], rawkv[64:128, :])

                def vtrans(ss):
                    for st4 in range(n_qt):
                        st = ss * n_qt + st4
                        pst = pvpool.tile([128, 128], BF16, tag="pv",
                                          name="pst")
                        nc.tensor.transpose(
                            pst[:, :64], vt_b[:, st * 128:(st + 1) * 128],
                            ident[:64, :64])
                        nc.vector.tensor_copy(v_b[:, st, :64], pst[:, :64])

                # deferred softmax normalization: emitted after the next
                # head-pair's first scores so DVE/GpSimd latency is covered
                pending_norm = []

                def flush_norm():
                    while pending_norm:
                        psv, h, s0 = pending_norm.pop(0)
                        zrow = wpool.tile([1, sup], F32, tag="zrow")
                        nc.vector.tensor_copy(zrow[:], psv[64:65, :])
                        recip = wpool.tile([1, sup], F32, tag="recip")
                        nc.vector.reciprocal_approx_fast(recip[:], zrow[:])
                        bcast = wpool.tile([64, sup], F32, tag="bcast")
                        nc.gpsimd.partition_broadcast(bcast[:], recip[:])
                        o = (h % 2) * 64
                        nc.vector.tensor_tensor(
                            attnT_b[o:o + 64, h // 2, s0:s0 + sup],
                            psv[0:64, :], bcast[:], mybir.AluOpType.mult)

                def oproj_quanta(ss):
                    """O-proj as a list of (2 matmul + 1 DVE copy) quanta,
                    interleaved into the next attention block's kt loop:
                    ACT exp (~118 G elem/s) is ~30% slower than the PE's
                    attention matmuls per tile, so pure-attention stretches
                    starve the PE; these quanta are the filler. pso rides
                    the "pj" ring (free during attention blocks)."""
                    quanta = []
                    osb_state = {}
                    for st4 in range(n_qt):
                        st = ss * n_qt + st4

                        def make(st=st):
                            def run_cs(cs):
                                if cs == 0:
                                    osb_state[st] = opool.tile(
                                        [128, hid], BF16, tag="osb",
                                        name="osb")
                                osb = osb_state[st]
                                pso = pjpool.tile([128, 512], F32, tag="pj",
                                                  name="pso")
                                for j in range(HD // 128):
                                    nc.tensor.matmul(
                                        pso[:],
                                        attnT_b[:, j,
                                                st * 128:(st + 1) * 128],
                                        wo_t[:, j, cs * 512:(cs + 1) * 512],
                                        start=(j == 0),
                                        stop=(j == HD // 128 - 1))
                                nc.vector.tensor_copy(
                                    osb[:, cs * 512:(cs + 1) * 512], pso[:])
                                if cs == n_cs - 1:
                                    row = b * s_sz + st * 128
                                    nc.sync.dma_start(
                                        out[row:row + 128, :], osb[:])
                            return run_cs
                        run_cs = make()
                        for cs in range(n_cs):
                            quanta.append((run_cs, cs))
                    return quanta

                def attention(ss, quanta=()):
                    quanta = list(quanta)
                    s0 = ss * sup
                    n_kt = (ss + 1) * n_qt
                    # spread the filler quanta across pv iterations, but
                    # keep them out of the first/last 2 iterations of each
                    # hp block: those cover the norm chain / psv handoff,
                    # and a quantum's DVE copy there would delay the norm
                    # multiplies that release the psv ring
                    eligible = max(0, (HQ // 2) * (n_kt - 4))
                    stride = max(1, -(-eligible // len(quanta))
                                 if quanta else 1)
                    it_count = [0]

                    def maybe_quantum(kt):
                        if kt < 2 or kt >= n_kt - 2:
                            return
                        it_count[0] += 1
                        if quanta and it_count[0] % stride == 0:
                            fn, cs = quanta.pop(0)
                            fn(cs)

                    for hp in range(HQ // 2):
                        heads = (2 * hp, 2 * hp + 1)
                        psvs = [pvpool.tile([128, sup], F32, tag="pv",
                                            name=f"psv{i}")
                                for i in range(2)]
                        pts = {}

                        def emit_score(kt):
                            k0 = kt * 128
                            dq = max(0, k0 - s0)
                            w = sup - dq
                            ps = scpool.tile([128, 2, sup], F32, tag="sc",
                                             name="ps")
                            pt = ptpool.tile([128, 2, sup], BF16, tag="pt")
                            for sub, h in enumerate(heads):
                                nc.tensor.matmul(
                                    ps[:, sub, 0:w],
                                    kt_b[:, h % 2, k0:k0 + 128],
                                    qt_b[:, h // 2, s0 + dq:s0 + sup],
                                    start=True, stop=True)
                            if k0 >= s0:
                                nc.vector.tensor_tensor(
                                    ps[:, :, 0:128], ps[:, :, 0:128],
                                    mask_t[:, None, :].to_broadcast(
                                        (128, 2, 128)),
                                    mybir.AluOpType.add)
                            nc.scalar.activation(
                                pt[:, :, dq:dq + w], ps[:, :, 0:w],
                                mybir.ActivationFunctionType.Exp,
                                scale=SCALE)
                            pts[kt] = (pt, dq)

                        def emit_pv(kt):
                            pt, dq = pts.pop(kt)
                            for sub in range(2):
                                nc.tensor.matmul(
                                    psvs[sub][:, dq:sup],
                                    v_b[:, kt, :],
                                    pt[:, sub, dq:sup],
                                    start=(kt == 0), stop=(kt == n_kt - 1),
                                    skip_group_check=True)

                        # depth-3 software pipeline: PE runs three score
                        # pairs ahead of each PV pair, so neither ACT exp
                        # latency nor the previous head-pair's deferred
                        # norm chain stalls the PE. flush_norm first: its
                        # DVE multiplies must lead the DVE queue so the
                        # psv ring releases before this hp's first pv.
                        flush_norm()
                        emit_score(0)
                        if n_kt > 1:
                            emit_score(1)
                        if n_kt > 2:
                            emit_score(2)
                        for kt in range(n_kt):
                            if kt + 3 < n_kt:
                                emit_score(kt + 3)
                            emit_pv(kt)
                            maybe_quantum(kt)
                        for sub, h in enumerate(heads):
                            pending_norm.append((psvs[sub], h, s0))
                    # norms lead the DVE queue ahead of the trailing
                    # quanta copies (the next block reads attnT)
                    flush_norm()
                    while quanta:
                        fn, cs = quanta.pop(0)
                        fn(cs)

                def oproj(ss):
                    for fn, cs in oproj_quanta(ss):
                        fn(cs)

                # lag-interleaved emission: the per-engine instruction order
                # is static, so put already-satisfiable attention/O work
                # between projection supertiles to cover RoPE latency.
                for ss in range(n_sup):
                    rawkv = proj_a(ss)
                    if ss >= 1:
                        vtrans(ss - 1)
                    proj_b(ss, rawkv)
                    if ss >= 1:
                        attention(ss - 1,
                                  oproj_quanta(ss - 2) if ss >= 2 else ())
                vtrans(n_sup - 1)
                attention(n_sup - 1,
                          oproj_quanta(n_sup - 2) if n_sup >= 2 else ())
                oproj(n_sup - 1)
    nc.compile()
    return nc


def _rope_tables_np(seq_len, dim, base=10000.0):
    inv_freq = 1.0 / (base ** (np.arange(0, dim, 2, dtype=np.float32) / dim))
    t = np.arange(seq_len, dtype=np.float32)
    freqs = np.outer(t, inv_freq)
    emb = np.concatenate([freqs, freqs], axis=-1)
    return np.cos(emb), np.sin(emb)


def host_prep(hidden_states, cos, sin, Wq, Wk, Wv, Wo, s_sz=None, hid=None,
              attention_mask=None):
    """Slice/transposes/casts -> per-core input maps."""
    b_sz = hidden_states.shape[0]
    s_sz = s_sz or hidden_states.shape[1]
    hid = hid or hidden_states.shape[2]
    bf = ml_dtypes.bfloat16

    supw = min(SSUP, s_sz)
    # [B*n_sup, 128, C, sup]: each supertile contiguous so its DMA moves in
    # 16 KiB-per-partition runs instead of 1 KiB strided packets
    hsT = np.ascontiguousarray(
        hidden_states.reshape(b_sz * s_sz // supw, supw, hid // 128, 128)
        .transpose(0, 3, 2, 1)).astype(bf)

    cosT = np.asarray(cos, np.float32).T          # [64, S]
    sinT = np.asarray(sin, np.float32).T
    cos2 = np.concatenate([cosT, cosT], axis=0)   # [128, S]
    # plain sin table (the signed permutation pi2 carries rotate_half signs)
    sinx = np.concatenate([sinT, sinT], axis=0)
    cos2 = np.ascontiguousarray(cos2).astype(bf)
    sinx = np.ascontiguousarray(sinx).astype(bf)

    if attention_mask is not None:
        # additive mask for the transposed diagonal block: M[k', q'] =
        # mask[q0+q', k0+k'] (identical for every diagonal block of a
        # causal mask, whatever its masked-value constant)
        maskd = np.ascontiguousarray(
            np.asarray(attention_mask, np.float32)[0, 0, :128, :128].T)
    else:
        kk, qq = np.meshgrid(np.arange(128), np.arange(128), indexing="ij")
        maskd = np.where(kk <= qq, 0.0, NEG).astype(np.float32)

    # lhsT of the rotate_half matmul: rot = pi2.T @ raw per 64-row head block
    # rot[d'] = -raw[d'+32] for d'<32, +raw[d'-32] for d'>=32
    pi64 = np.zeros((64, 64), np.float32)
    for r in range(32):
        pi64[r, r + 32] = 1.0       # row r feeds out col r+32 with +1
        pi64[r + 32, r] = -1.0      # row r+32 feeds out col r with -1
    pi2d = np.zeros((128, 128), np.float32)
    pi2d[:64, :64] = pi64
    pi2d[64:, 64:] = pi64
    pi2d = pi2d.astype(bf)

    def ptile(w):
        # [(C p), M] -> [p, C*M] so each partition's DMA is one long run
        rows, m = w.shape
        return np.ascontiguousarray(
            w.reshape(rows // 128, 128, m).transpose(1, 0, 2)
            .reshape(128, -1)).astype(bf)

    in_maps = []
    for c in range(NCORES):
        wq_c = ptile(Wq[:, c * HD:(c + 1) * HD])
        wkv_c = ptile(np.concatenate(
            [Wk[:, c * D:(c + 1) * D], Wv[:, c * D:(c + 1) * D]], axis=1))
        wo_c = ptile(Wo[c * HD:(c + 1) * HD, :])
        in_maps.append({
            "hsT": hsT, "wq": wq_c, "wkv": wkv_c,
            "wo": wo_c, "cos2": cos2, "sinx": sinx, "maskd": maskd,
            "pi2d": pi2d,
        })
    return in_maps


def kernel_run(hidden_states, cos, sin, attention_mask, Wq, Wk, Wv, Wo,
               **spmd_kwargs):
    from concourse.bass_utils import run_bass_kernel_spmd

    hidden_states = np.asarray(hidden_states, np.float32)
    in_maps = host_prep(hidden_states, cos, sin,
                        np.asarray(Wq, np.float32), np.asarray(Wk, np.float32),
                        np.asarray(Wv, np.float32), np.asarray(Wo, np.float32),
                        attention_mask=attention_mask)
    nc = build_nc()
    res = run_bass_kernel_spmd(nc, in_maps, core_ids=list(range(NCORES)),
                               **spmd_kwargs)
    acc = np.zeros((B * S, HID), np.float32)
    for r in res.results:
        acc += r["out"].astype(np.float32)
    return acc.reshape(B, S, HID), res


def kernel(hidden_states, cos, sin, attention_mask, Wq, Wk, Wv, Wo):
    out, _ = kernel_run(hidden_states, cos, sin, attention_mask,
                        Wq, Wk, Wv, Wo)
    return out


if __name__ == "__main__":
    pass
